# revision 4
# baseline (speedup 1.0000x reference)
"""Trainium2 Bass kernel for the 5-layer dilated sparse-conv encoder (fp8).

Network (per batch): 1ch -> [3x3x3 dil1] -> 2ch -> [3x3x3 dil2] -> 2ch
-> [3x3x3 dil4] -> 2ch -> [3x3x3 dil2] -> 2ch -> [1x1x1] -> sigmoid,
with relu+occupancy-mask after each hidden conv and mask after sigmoid.

Sharding: 8 cores = 2 batches x 4 z-slabs of 48 planes. Each core gets a
66-plane input slab (z halo 9) and computes its 48 output planes with no
cross-core communication.

Per-core algorithm: contraction over z on the TensorEngine in fp8e4m3.
Activations live in SBUF as [2ch*64 partitions, y, 208] fp8 where the
64-row z-window is [z0-8, z1+8) and x carries 8 zero-pad columns each
side (keeping the row pitch 16B-aligned). A conv layer needs 6 tensor
ops per output tile: the (dy=-1,dx)+(dy=+1,dx) tap pairs are packed
two-per-matmul with DoubleRow perf mode - the pair dimension's rhs
access-pattern stride is 2*dil*208 bytes (DoubleRow requires 16B-aligned
pair strides; odd strides hang the PE) - and the three dy=0 taps run as
plain fp8 matmuls. The 3 dz taps and both channels fold into banded
weight matrices [K, 9, 128] built host-side. Zero x/y borders make every
tap's output rectangle full, so no border clipping is needed.
relu+mask+fp8-quantize is one fused scalar_tensor_tensor DVE op per
2-bank PSUM tile; the occupancy mask is precomputed host-side and DMA'd.
"""

import os
import sys

import numpy as np


def _ensure_import_path():
    for p in ("/opt/trn_rl_repo", "/root/.axon_site/_ro/trn_rl_repo"):
        if os.path.isdir(p) and p not in sys.path:
            sys.path.insert(0, p)


_ensure_import_path()

import ml_dtypes  # noqa: E402
import concourse.mybir as mybir  # noqa: E402
import concourse.tile as tile  # noqa: E402
from concourse import bacc, bass_utils  # noqa: E402

B, D = 2, 192
ZS = 48  # z planes per core
HZ = 9  # input z halo
ZIN = ZS + 2 * HZ  # 66 input planes per core
XP = 8  # x zero-pad columns each side (keeps ROWP 16B-aligned)
YP = 4  # top y zero-pad rows in each tile
ROWP = D + 2 * XP  # 208 row pitch
# (dilation, valid z-window in 64-coords) per layer
LAYERS = [(1, 0, 64), (2, 2, 62), (4, 6, 58), (2, 8, 56)]
V5 = (8, 56)

# DoubleRow dy-pairs (aligned pair stride 2*d*ROWP) + plain dy=0 taps
TPAIR = [((-1, -1), (1, -1)), ((-1, 0), (1, 0)), ((-1, 1), (1, 1))]
TSING = [(0, -1), (0, 0), (0, 1)]

YBLK = 16  # output-y rows per wavefront block
FP8 = ml_dtypes.float8_e4m3fn


def _tap_slots():
    """9 taps -> band slot index: slots 2t,2t+1 = pair t's (A,B); 6+s = single."""
    out = []
    for t, (A, Bt) in enumerate(TPAIR):
        out.append((A, 2 * t))
        out.append((Bt, 2 * t + 1))
    for s, T in enumerate(TSING):
        out.append((T, 6 + s))
    return out


def _build_bands(W1, W2, W3, W4, W5):
    """Banded lhsT matrices [K, 9, 128] per layer (fp8). k folds (ci, z');
    m folds (co, z); dim1 is the band slot (3 DR pairs + 3 singles)."""
    Ws = [np.asarray(w, np.float32) for w in (W1, W2, W3, W4)]
    slots = _tap_slots()
    out = {}
    # L1: [66, 9, 128] (Cin=1): k = zr + 1 + dz, m = co*64 + zr
    b1 = np.zeros((ZIN, 9, 128), np.float32)
    zr = np.arange(64)
    for (dy, dx), sl in slots:
        for co in range(2):
            for dz in (-1, 0, 1):
                b1[zr + 1 + dz, sl, co * 64 + zr] = Ws[0][
                    co, 0, dz + 1, dy + 1, dx + 1
                ]
    out["b1"] = b1.astype(FP8)
    # L2..L4: [128, 9, 128]: k = ci*64 + zv + d*dz, m = co*64 + zv
    for li, (d, a, b) in enumerate(LAYERS[1:], start=2):
        w = Ws[li - 1]
        zv = np.arange(a, b)
        bb = np.zeros((128, 9, 128), np.float32)
        for (dy, dx), sl in slots:
            for co in range(2):
                for ci in range(2):
                    for dz in (-1, 0, 1):
                        bb[ci * 64 + zv + d * dz, sl, co * 64 + zv] = w[
                            co, ci, dz + 1, dy + 1, dx + 1
                        ]
        out[f"b{li}"] = bb.astype(FP8)
    # L5: [128, 128]
    w5 = np.asarray(W5, np.float32)
    b5 = np.zeros((128, 128), np.float32)
    zv = np.arange(V5[0], V5[1])
    for co in range(2):
        for ci in range(2):
            b5[ci * 64 + zv, co * 64 + zv] = w5[co, ci, 0, 0, 0]
    out["b5"] = b5.astype(FP8)
    return out


def _pair_ap(src, K, sy0, rows, sx0, cols, delta):
    """4D rhs AP [K, 2, rows, cols]: pair dim stride = delta elements."""
    ap = src[0:K, sy0 : sy0 + rows, sx0 : sx0 + cols].unsqueeze(1)
    ap.ap[1] = [delta, 2]
    return ap


def build_program():
    dt8 = mybir.dt.float8e4
    f32 = mybir.dt.float32
    DR = mybir.MatmulPerfMode.DoubleRow
    nc = bacc.Bacc("TRN2", target_bir_lowering=False, debug=False)

    xslab = nc.dram_tensor("xslab", [ZIN, D, ROWP], dt8, kind="ExternalInput")
    mslab = nc.dram_tensor("mslab", [64, D, D], dt8, kind="ExternalInput")
    b1d = nc.dram_tensor("b1", [ZIN, 9, 128], dt8, kind="ExternalInput")
    b2d = nc.dram_tensor("b2", [128, 9, 128], dt8, kind="ExternalInput")
    b3d = nc.dram_tensor("b3", [128, 9, 128], dt8, kind="ExternalInput")
    b4d = nc.dram_tensor("b4", [128, 9, 128], dt8, kind="ExternalInput")
    b5d = nc.dram_tensor("b5", [128, 128], dt8, kind="ExternalInput")
    prob_o = nc.dram_tensor("prob_o", [ZS, D, D], f32, kind="ExternalOutput")
    regr_o = nc.dram_tensor("regr_o", [ZS, D, D], f32, kind="ExternalOutput")

    # Skewed y-wavefront: layer l's computed frontier leads the output by
    # h_l rows. Block b computes rows [F(h,b-1), F(h,b)) of each layer.
    # t1..t3 are full-height (fp8 is cheap), so only xt needs a rolling
    # window with a small tail copy between blocks.
    HLEAD = {"xt": HZ, "t1": 8, "t2": 6, "t3": 2, "t4": 0}
    DCONS = {"xt": 1, "t1": 2, "t2": 4, "t3": 2, "t4": 0}

    def F(h, b):
        return 0 if b < 0 else min(YBLK * (b + 1) + h, D)

    NB = D // YBLK

    with tile.TileContext(nc) as tc:
        with (
            tc.tile_pool(name="wpool", bufs=1) as wp,
            tc.tile_pool(name="act", bufs=1) as ap,
            tc.tile_pool(name="mkp", bufs=2) as mkp,
            tc.tile_pool(name="otp", bufs=2) as otp,
            tc.tile_pool(name="ps", bufs=3, space="PSUM") as ps,
        ):
            b1t = wp.tile([ZIN, 9, 128], dt8)
            b2t = wp.tile([128, 9, 128], dt8)
            b3t = wp.tile([128, 9, 128], dt8)
            b4t = wp.tile([128, 9, 128], dt8)
            b5t = wp.tile([128, 128], dt8)
            # b1 on the HWDGE path (needed first, ahead of block 0's input);
            # the rest via SWDGE so they don't queue ahead of it
            nc.sync.dma_start(b1t[:], b1d[:])
            for t, dram in ((b2t, b2d), (b3t, b3d), (b4t, b4d), (b5t, b5d)):
                nc.gpsimd.dma_start(t[:], dram[:])

            XEXT = YP + YBLK + HZ + 2 * DCONS["xt"] + 4  # rolling xt rows
            xt = ap.tile([ZIN, XEXT, ROWP], dt8, tag="xt")
            t1 = ap.tile([128, YP + D + 4, ROWP], dt8, tag="t1")
            t2 = ap.tile([128, YP + D + 4, ROWP], dt8, tag="t2")
            t3 = ap.tile([128, YP + D + 4, ROWP], dt8, tag="t3")
            t4 = ap.tile([128, YP + YBLK + 4, ROWP], dt8, tag="t4")

            # zero only the borders that taps read: x-pad strips, top y-pad
            # rows, bottom y-pad rows; spread across engines so they run in
            # parallel and don't delay block 0's input DMA / first matmuls
            nc.vector.memset(xt[:, :, 0:XP], 0.0)
            nc.vector.memset(xt[:, :, XP + D : ROWP], 0.0)
            nc.vector.memset(xt[:, 0:YP, XP : XP + D], 0.0)
            for t, eng in ((t1, nc.gpsimd), (t2, nc.gpsimd), (t3, nc.vector)):
                eng.memset(t[:, :, 0:XP], 0.0)
                eng.memset(t[:, :, XP + D : ROWP], 0.0)
                eng.memset(t[:, 0:YP, XP : XP + D], 0.0)
                eng.memset(t[:, YP + D : YP + D + 4, XP : XP + D], 0.0)

            def origin(name, b):
                if name in ("t1", "t2", "t3"):
                    return 0
                h, dc = HLEAD[name], DCONS[name]
                return 0 if b == 0 else F(h, b - 1) - 2 * dc

            chain = (
                ("xt", xt, ZIN, b1t, 1, "t1", t1),
                ("t1", t1, 128, b2t, 2, "t2", t2),
                ("t2", t2, 128, b3t, 4, "t3", t3),
                ("t3", t3, 128, b4t, 2, "t4", t4),
            )

            for b in range(NB):
                # xt tail: last 2 computed rows -> tile rows [YP, YP+2)
                if b > 0:
                    s0 = F(HZ, b - 1) - 2 - origin("xt", b - 1) + YP
                    nc.vector.tensor_copy(
                        xt[:, YP : YP + 2, :], xt[:, s0 : s0 + 2, :]
                    )

                # new input rows
                i0, i1 = F(HZ, b - 1), F(HZ, b)
                r0 = i0 - origin("xt", b) + YP
                nc.sync.dma_start(xt[:, r0 : r0 + i1 - i0, :], xslab[:, i0:i1, :])
                if b == NB - 1:
                    rr = D - origin("xt", b) + YP
                    nc.vector.memset(xt[:, rr : rr + 2, :], 0.0)

                # mask for this block's union of layer windows (fp8 0/1,
                # precomputed host-side)
                m0, m1 = YBLK * b, min(YBLK * b + YBLK + 8, D)
                mk = mkp.tile([128, YBLK + 8, D], dt8, tag="mk")
                nc.sync.dma_start(mk[0:64, 0 : m1 - m0, :], mslab[:, m0:m1, :])
                nc.sync.dma_start(mk[64:128, 0 : m1 - m0, :], mslab[:, m0:m1, :])

                for sname, stile, K, bt, d, dname, dtile in chain:
                    w0, w1 = F(HLEAD[dname], b - 1), F(HLEAD[dname], b)
                    so = origin(sname, b)
                    do = origin(dname, b)
                    for g0 in range(w0, w1, 16):
                        g1 = min(g0 + 16, w1)
                        accs = []
                        for _xi in range(3):
                            acc = ps.tile([128, 16, 64], f32, tag="cacc", bufs=3)
                            accs.append(acc)
                        # 3 DR dy-pairs (delta = 2*d*ROWP, 16B-aligned),
                        # then 3 plain dy=0 taps
                        for tp in range(6):
                            dyA, dxA = TPAIR[tp][0] if tp < 3 else TSING[tp - 3]
                            for xi in range(3):
                                for h in range(0, g1 - g0, 8):
                                    ys = g0 + h
                                    ye = min(ys + 8, g1)
                                    sy0 = ys + dyA * d - so + YP
                                    sx0 = XP + xi * 64 + dxA * d
                                    if tp < 3:
                                        rhs = _pair_ap(
                                            stile, K, sy0, ye - ys, sx0, 64,
                                            2 * d * ROWP,
                                        )
                                        lhs = bt[0:K, 2 * tp : 2 * tp + 2, :]
                                        pm = DR
                                    else:
                                        rhs = stile[
                                            0:K, sy0 : sy0 + ye - ys,
                                            sx0 : sx0 + 64,
                                        ]
                                        lhs = bt[0:K, 3 + tp, :]
                                        pm = None
                                    nc.tensor.matmul(
                                        accs[xi][:, h : h + (ye - ys), :],
                                        lhs,
                                        rhs,
                                        start=(tp == 0),
                                        stop=(tp == 5),
                                        perf_mode=pm,
                                    )
                        # fused relu+mask+fp8 quantize per 1-bank acc tile
                        for xi in range(3):
                            nc.vector.scalar_tensor_tensor(
                                dtile[
                                    :,
                                    g0 - do + YP : g1 - do + YP,
                                    XP + xi * 64 : XP + (xi + 1) * 64,
                                ],
                                accs[xi][:, 0 : g1 - g0, :],
                                0.0,
                                mk[:, g0 - m0 : g1 - m0, xi * 64 : (xi + 1) * 64],
                                op0=mybir.AluOpType.max,
                                op1=mybir.AluOpType.mult,
                            )

                # L5: 1x1 conv + sigmoid + mask + store, in 8-row groups
                w5 = (F(0, b - 1), F(0, b))
                o4 = origin("t4", b)
                for ys in range(w5[0], w5[1], 8):
                    ye = min(ys + 8, w5[1])
                    ot = otp.tile([128, 8, D], f32, tag="ot")
                    for ps0 in range(ys, ye, 2):
                        ps1 = min(ps0 + 2, ye)
                        acc = ps.tile([128, 2, D], f32, tag="l5acc", bufs=2)
                        nc.tensor.matmul(
                            acc[:, 0 : ps1 - ps0, :],
                            b5t[:, :],
                            t4[:, ps0 - o4 + YP : ps1 - o4 + YP, XP : XP + D],
                            start=True,
                            stop=True,
                        )
                        nc.scalar.activation(
                            ot[:, ps0 - ys : ps1 - ys, :],
                            acc[:, 0 : ps1 - ps0, :],
                            mybir.ActivationFunctionType.Sigmoid,
                        )
                    nc.gpsimd.tensor_tensor(
                        ot[:, 0 : ye - ys, :],
                        ot[:, 0 : ye - ys, :],
                        mk[:, ys - m0 : ye - m0, :],
                        op=mybir.AluOpType.mult,
                    )
                    nc.sync.dma_start(
                        prob_o[:, ys:ye, :], ot[8:56, 0 : ye - ys, :]
                    )
                    nc.sync.dma_start(
                        regr_o[:, ys:ye, :], ot[72:120, 0 : ye - ys, :]
                    )

    nc.compile()
    return nc


_prog_cache = {}


def make_in_maps(data, W1, W2, W3, W4, W5):
    bands = _build_bands(W1, W2, W3, W4, W5)
    data = np.asarray(data, np.float32)
    d8 = data.astype(FP8)
    # keep the occupancy pattern exact: values that quantize to zero get the
    # smallest fp8 subnormal with the right sign
    tiny = (data != 0) & (d8.astype(np.float32) == 0)
    d8[tiny] = np.copysign(2.0**-9, data[tiny]).astype(FP8)
    dpad = np.zeros((B, D + 2 * HZ, D, ROWP), FP8)
    dpad[:, HZ : HZ + D, :, XP : XP + D] = d8
    m8 = (data != 0).astype(FP8)
    mpad = np.zeros((B, D + 2 * HZ, D, D), FP8)
    mpad[:, HZ : HZ + D] = m8
    in_maps = []
    for c in range(8):
        bi, s = c // 4, c % 4
        in_maps.append(
            dict(
                xslab=np.ascontiguousarray(dpad[bi, s * ZS : s * ZS + ZIN]),
                mslab=np.ascontiguousarray(
                    mpad[bi, s * ZS + 1 : s * ZS + 65]
                ),
                **bands,
            )
        )
    return in_maps


def kernel(data, W1, W2, W3, W4, W5):
    _ensure_import_path()
    if "nc" not in _prog_cache:
        _prog_cache["nc"] = build_program()
    nc = _prog_cache["nc"]

    in_maps = make_in_maps(data, W1, W2, W3, W4, W5)
    res = bass_utils.run_bass_kernel_spmd(nc, in_maps, list(range(8))).results

    prob = np.zeros((B, 1, D, D, D), np.float32)
    regr = np.zeros((B, 1, D, D, D), np.float32)
    for c in range(8):
        bi, s = c // 4, c % 4
        prob[bi, 0, s * ZS : (s + 1) * ZS] = res[c]["prob_o"]
        regr[bi, 0, s * ZS : (s + 1) * ZS] = res[c]["regr_o"]
    return (prob, regr)
